# revision 4
# baseline (speedup 1.0000x reference)
"""Multi-head linear attention on Trainium2 — 8-core SPMD, batch+head sharded.

Full-tensor contract: kernel(**inputs) takes the complete Q/K/V
[4, 4096, 1024] f32 arrays, internally shards them across 8 NeuronCores
(core c -> batch c//2, heads 8*(c%2) .. 8*(c%2)+8, i.e. a contiguous
512-column slice of the embedding dim), runs one Bass kernel per core,
and reassembles the full [4, 4096, 1024] f32 output.

Per-core math (H=8 local heads, D=64, L=4096):
    phi = sigmoid(0.6053*x - 4.102)
    kv_ext[h] = phi_K[h]^T @ [V[h] | 1]     # [64, 65], f32 PSUM accum
    numden[h] = phi_Q[h] @ kv_ext[h]        # [L, 65]
    out[h]    = numden[h][:, :64] / numden[h][:, 64:65]

All device I/O is fp16 (host casts f32 -> fp16 in, fp16 -> f32 out;
matmul accumulation stays f32 in PSUM): 12.5 MiB loads + 4 MiB stores
per core.

The 8 heads form 4 PAIRS, processed as a 4-deep pipeline: pair g's
kv-accumulation streams while pair g-1's Q phase computes, so only the
last pair's Q phase (~2 pieces) runs after the final loads.  Host
staging per core:
  Q: transposed [512, L] fp16, pair-major rows; loaded whole-pair into
     resident SBUF on the Vector HWDGE queue with 1 KiB descriptors so
     round-robin across DMA queues cannot starve the K/V streams.
  K: pair-major [4L, 128] fp16; partition lines carry 4 consecutive
     L-rows (1 KiB descriptors), K on the Sync HWDGE queue.
  V: pair-major [4L, 130] fp16 rows [V_pair(128) | 1 | 1] — ones baked
     in on the host, so a single [128,130] matmul per 128-row chunk
     accumulates kv AND k_sum with no wasted columns; 4-row partition
     lines give 1040 B descriptors; V on the Pool SWDGE queue.
  O: [128, 4L] fp16, one 4 KiB-descriptor store per (pair, 2048-q)
     piece on the Scalar queue.

kv for a pair accumulates in one [128, 130] PSUM tile (head0 rows 0:64,
head1 rows 64:128, k_sum in col 128); the Q-phase matmul multiplies a
128-q block of phi_Q^T against a block-diagonal [128, 130] kv operand,
yielding both heads' num|den.  Division runs on VectorE batched 3
q-blocks per PSUM bank: one strided reciprocal + one 4-D-strided
broadcast multiply.
"""

import numpy as np

B = 4
L = 4096
E = 1024
NH = 8            # heads per core
D = 64
W = D + 1         # head block width incl. ones/den column
EC = NH * D       # 512 embedding columns per core
P = 128
G = 4             # head pairs, stacked along rows (pipeline depth)
GC = EC // G      # 128 columns per pair
SUB = 4           # L-rows per partition line
VW = 2 * W        # 130: pair block width in V staging / kv tiles
NT = L // (P * SUB)   # 8 tiles (512 L-rows) per pair
TBS = 2           # tiles per DMA batch
NBS = NT // TBS   # 4 batches per tensor per pair
QB = 2048         # q columns per Q-phase piece
NQB = L // QB     # 2 pieces per pair
N_CORES = 8

_CACHE = {}


def _build_nc():
    from contextlib import ExitStack

    import concourse.bacc as bacc
    import concourse.bass as bass
    import concourse.mybir as mybir
    import concourse.tile as tile

    f32 = mybir.dt.float32
    f16 = mybir.dt.float16
    SIG = mybir.ActivationFunctionType.Sigmoid

    nc = bacc.Bacc("TRN2", target_bir_lowering=False, debug=False)
    Q = nc.dram_tensor("Q", [EC, L], f16, kind="ExternalInput").ap()
    K = nc.dram_tensor("K", [G * L, GC], f16, kind="ExternalInput").ap()
    V = nc.dram_tensor("V", [G * L, VW], f16, kind="ExternalInput").ap()
    O = nc.dram_tensor("O", [P, G * L], f16, kind="ExternalOutput").ap()

    with tile.TileContext(nc) as tc, ExitStack() as ctx:
        singles = ctx.enter_context(tc.tile_pool(name="singles", bufs=1))
        ld = ctx.enter_context(tc.tile_pool(name="ld", bufs=3))
        vb = ctx.enter_context(tc.tile_pool(name="vb", bufs=3))
        ph = ctx.enter_context(tc.tile_pool(name="ph", bufs=3))
        qt = ctx.enter_context(tc.tile_pool(name="qt", bufs=3))
        rcp = ctx.enter_context(tc.tile_pool(name="rcp", bufs=6))
        ob = ctx.enter_context(tc.tile_pool(name="ob", bufs=3))
        pn = ctx.enter_context(tc.tile_pool(name="pn", bufs=4, space="PSUM"))
        pk = ctx.enter_context(tc.tile_pool(name="pk", bufs=1, space="PSUM"))

        sig_bias = singles.tile([P, 1], f32)
        nc.vector.memset(sig_bias, -4.102)

        # Block-diagonal kv operand per head pair: rows 0:64 cols 0:65 hold
        # kv_ext of the even head, rows 64:128 cols 65:130 the odd head.
        kv_bd = singles.tile([P, G, VW], f16)
        nc.vector.memset(kv_bd, 0.0)

        # Full-bank PSUM tiles (no matmul output may straddle a bank).
        kv_ps = [pk.tile([P, 512], f32, tag=f"kv{g}", name=f"kv{g}")
                 for g in range(G)]

        # Whole-Q resident buffers, one per head pair (8 KiB/partition).
        q_raw = [singles.tile([P, L], f16, tag=f"qr{g}", name=f"qr{g}")
                 for g in range(G)]

        def emit_q_load(g):
            # 512-element inner dim forces 1 KiB descriptors so the Q
            # stream cannot out-compete K/V in DMA round-robin.
            nc.scalar.dma_start(
                out=q_raw[g].rearrange("p (k e) -> p k e", e=512),
                in_=Q[g * P:(g + 1) * P, :].rearrange(
                    "p (k e) -> p k e", e=512),
            )

        def emit_kv_batch(g, ib):
            """Load K batch + V batch, sigmoid, accumulate pair kv."""
            rows = slice(g * L + ib * TBS * P * SUB,
                         g * L + (ib + 1) * TBS * P * SUB)
            k_raw = ld.tile([P, TBS, SUB, GC], f16, tag="kraw", name="k_raw")
            nc.sync.dma_start(
                out=k_raw,
                in_=K[rows, :].rearrange("(t p s) e -> p t s e", p=P, s=SUB),
            )
            phiK = ph.tile([P, TBS, SUB, GC], f16, tag="phiK", name="phiK")
            nc.scalar.activation(
                out=phiK, in_=k_raw, func=SIG, bias=sig_bias, scale=0.6053
            )
            vt = vb.tile([P, TBS, SUB, VW], f16, tag="v", name="v")
            nc.gpsimd.dma_start(
                out=vt,
                in_=V[rows, :].rearrange("(t p s) e -> p t s e", p=P, s=SUB),
            )
            for t in range(TBS):
                for s in range(SUB):
                    nc.tensor.matmul(
                        out=kv_ps[g][:, 0:VW],
                        lhsT=phiK[:, t, s, :],
                        rhs=vt[:, t, s, :],
                        start=(ib == 0 and t == 0 and s == 0),
                        stop=(ib == NBS - 1 and t == TBS - 1
                              and s == SUB - 1),
                    )

        def emit_kv_finish(g):
            """Pack the pair's kv PSUM tile into the block-diag operand."""
            nc.vector.tensor_copy(
                out=kv_bd[0:D, g, 0:D], in_=kv_ps[g][0:D, 0:D])
            nc.vector.tensor_copy(
                out=kv_bd[0:D, g, D:W], in_=kv_ps[g][0:D, 2 * D:2 * D + 1])
            nc.vector.tensor_copy(
                out=kv_bd[D:P, g, W:W + D], in_=kv_ps[g][D:P, D:2 * D])
            nc.vector.tensor_copy(
                out=kv_bd[D:P, g, W + D:VW], in_=kv_ps[g][D:P, 2 * D:2 * D + 1])

        def emit_q_piece(g, qb):
            """sigmoid 2048 q + 16 matmuls + batched div + one store."""
            qtT = qt.tile([P, QB], f16, tag="qtT", name="qtT")
            nc.scalar.activation(
                out=qtT, in_=q_raw[g][:, qb * QB:(qb + 1) * QB],
                func=SIG, bias=sig_bias, scale=0.6053,
            )
            out_t = ob.tile([P, QB], f16, tag="outt", name="out_t")
            nqk = QB // P               # 16
            qk = 0
            while qk < nqk:
                nb = min(3, nqk - qk)   # triples, remainder 1
                num = pn.tile([P, 3, VW], f32, tag="num", name="num")
                for i in range(nb):
                    nc.tensor.matmul(
                        out=num[:, i, :],
                        lhsT=qtT[:, (qk + i) * P:(qk + i + 1) * P],
                        rhs=kv_bd[:, g, :],
                    )
                r = rcp.tile([P, 3, 2], f32, tag="r", name="r")
                den = bass.AP(
                    tensor=num.tensor, offset=num.offset + D,
                    ap=[num.ap[0], [VW, nb], [W, 2]],
                )
                nc.vector.reciprocal(out=r[:, 0:nb, :], in_=den)
                nums = bass.AP(
                    tensor=num.tensor, offset=num.offset,
                    ap=[num.ap[0], [VW, nb], [W, 2], [1, D]],
                )
                r_bc = bass.AP(
                    tensor=r.tensor, offset=r.offset,
                    ap=[r.ap[0], [2, nb], [1, 2], [0, D]],
                )
                nc.vector.tensor_tensor(
                    out=out_t[:, qk * P:(qk + nb) * P].rearrange(
                        "p (a b d) -> p a b d", a=nb, b=2),
                    in0=nums, in1=r_bc, op=mybir.AluOpType.mult,
                )
                qk += nb
            obase = g * L + qb * QB
            nc.scalar.dma_start(out=O[:, obase:obase + QB], in_=out_t)

        # ---- software-pipelined emission: pair g's kv streams while
        # pair g-1's Q phase computes ----
        emit_q_load(0)
        emit_q_load(1)
        for ib in range(NBS):
            emit_kv_batch(0, ib)
        emit_kv_finish(0)
        for g in range(1, G):
            if g + 1 < G:
                emit_q_load(g + 1)
            for ib in range(NBS):
                emit_kv_batch(g, ib)
                if ib % 2 == 1:
                    emit_q_piece(g - 1, (ib - 1) // 2)
            emit_kv_finish(g)
        for qb in range(NQB):
            emit_q_piece(G - 1, qb)

    nc.compile()
    return nc


def _get_nc():
    if "nc" not in _CACHE:
        _CACHE["nc"] = _build_nc()
    return _CACHE["nc"]


def _shard_q(arr):
    """Full [B, L, E] f32 -> per-core transposed [512, L] fp16 slices."""
    out = []
    for c in range(N_CORES):
        b, g = divmod(c, 2)
        out.append(np.ascontiguousarray(
            arr[b, :, g * EC:(g + 1) * EC].T.astype(np.float16)))
    return out


def _shard_k(arr):
    """Full [B, L, E] f32 -> per-core pair-major [4L, 128] fp16."""
    out = []
    for c in range(N_CORES):
        b, g = divmod(c, 2)
        sl = arr[b, :, g * EC:(g + 1) * EC].astype(np.float16)
        out.append(np.ascontiguousarray(
            np.concatenate([sl[:, pg * GC:(pg + 1) * GC] for pg in range(G)],
                           axis=0)))
    return out


def _shard_v(arr):
    """Full [B, L, E] f32 -> per-core pair-major [4L, 130] fp16 with
    ones baked into columns 128:130."""
    out = []
    for c in range(N_CORES):
        b, g = divmod(c, 2)
        sl = arr[b, :, g * EC:(g + 1) * EC].astype(np.float16)
        st = np.ones((G * L, VW), dtype=np.float16)
        for pg in range(G):
            st[pg * L:(pg + 1) * L, 0:P] = sl[:, pg * P:(pg + 1) * P]
        out.append(st)
    return out


def _unshard_o(o):
    """Per-core [128, 4L] fp16 -> [L, EC] f32 core slice."""
    blocks = o.reshape(P, G, NQB, QB // P, P)   # [p, pg, qb, qk, e]
    # q = qb*QB + qk*P + p
    perm = blocks.transpose(1, 2, 3, 0, 4).reshape(G, L, P)
    return np.concatenate(list(perm), axis=1).astype(np.float32)


def run_sharded(in_maps, trace=False, trace_cores=None):
    from concourse.bass_utils import run_bass_kernel_spmd

    nc = _get_nc()
    kwargs = {}
    if trace:
        kwargs = dict(trace=True, trace_cores=trace_cores or [0])
    return run_bass_kernel_spmd(nc, in_maps, core_ids=list(range(N_CORES)), **kwargs)


def kernel(**inputs):
    Q = np.asarray(inputs["Q"], dtype=np.float32)
    K = np.asarray(inputs["K"], dtype=np.float32)
    V = np.asarray(inputs["V"], dtype=np.float32)
    qs, ks, vs = _shard_q(Q), _shard_k(K), _shard_v(V)
    in_maps = [{"Q": qs[c], "K": ks[c], "V": vs[c]} for c in range(N_CORES)]
    res = run_sharded(in_maps)
    out = np.empty((B, L, E), dtype=np.float32)
    for c in range(N_CORES):
        b, g = divmod(c, 2)
        out[b, :, g * EC:(g + 1) * EC] = _unshard_o(res.results[c]["O"])
    return out


# revision 5
# speedup vs baseline: 1.1162x; 1.1162x over previous
"""Multi-head linear attention on Trainium2 — 8-core SPMD, batch+head sharded.

Full-tensor contract: kernel(**inputs) takes the complete Q/K/V
[4, 4096, 1024] f32 arrays, internally shards them across 8 NeuronCores
(core c -> batch c//2, heads 8*(c%2) .. 8*(c%2)+8, i.e. a contiguous
512-column slice of the embedding dim), runs one Bass kernel per core,
and reassembles the full [4, 4096, 1024] f32 output.

Per-core math (H=8 local heads, D=64, L=4096):
    phi = sigmoid(0.6053*x - 4.102)
    kv_ext[h] = phi_K[h]^T @ [V[h] | 1]     # [64, 65], f32 PSUM accum
    numden[h] = phi_Q[h] @ kv_ext[h]        # [L, 65]
    out[h]    = numden[h][:, :64] / numden[h][:, 64:65]

All device I/O is fp16 (host casts f32 -> fp16 in, fp16 -> f32 out;
matmul accumulation stays f32 in PSUM): 12.5 MiB loads + 4 MiB stores
per core.

The 8 heads form 4 PAIRS, processed as a 4-deep pipeline: pair g's
kv-accumulation streams while pair g-1's Q phase computes.  phi_Q for a
whole pair is precomputed into a resident SBUF buffer as soon as its Q
slice lands, so Q-phase pieces are pure PE -> VectorE -> store chains
with no Scalar-engine dependency.  Queue assignment keeps every
dispatch stream stall-free: K+V interleave on the Sync HWDGE queue
(K 1 KiB / V 1040 B descriptors), Q loads ride the Scalar queue ahead
of the sigmoids (1 KiB descriptors), and O stores sit alone on the
Pool SWDGE queue where their long data-waits block nothing.

Host staging per core:
  Q: transposed [512, L] fp16, pair-major rows.
  K: pair-major [4L, 128] fp16, 4 consecutive L-rows per partition line.
  V: pair-major [4L, 130] fp16 rows [V_pair(128) | 1 | 1] — ones baked
     in on the host, so a single [128,130] matmul per 128-row chunk
     accumulates kv AND k_sum with no wasted columns.
  O: [128, 4L] fp16.

kv for a pair accumulates in one [128, 130] PSUM tile (head0 rows 0:64,
head1 rows 64:128, k_sum in col 128); the Q-phase matmul multiplies a
128-q block of phi_Q^T against a block-diagonal [128, 130] kv operand,
yielding both heads' num|den.  Division runs on VectorE batched 3
q-blocks per PSUM bank: one strided reciprocal + one 4-D-strided
broadcast multiply.
"""

import numpy as np

B = 4
L = 4096
E = 1024
NH = 8            # heads per core
D = 64
W = D + 1         # head block width incl. ones/den column
EC = NH * D       # 512 embedding columns per core
P = 128
G = 4             # head pairs, stacked along rows (pipeline depth)
GC = EC // G      # 128 columns per pair
SUB = 4           # L-rows per partition line
VW = 2 * W        # 130: pair block width in V staging / kv tiles
NT = L // (P * SUB)   # 8 tiles (512 L-rows) per pair
TBS = 2           # tiles per DMA batch
NBS = NT // TBS   # 4 batches per tensor per pair
QB = 2048         # q columns per Q-phase piece
NQB = L // QB     # 2 pieces per pair
N_CORES = 8

_CACHE = {}


def _build_nc():
    from contextlib import ExitStack

    import concourse.bacc as bacc
    import concourse.bass as bass
    import concourse.mybir as mybir
    import concourse.tile as tile

    f32 = mybir.dt.float32
    f16 = mybir.dt.float16
    SIG = mybir.ActivationFunctionType.Sigmoid

    nc = bacc.Bacc("TRN2", target_bir_lowering=False, debug=False)
    Q = nc.dram_tensor("Q", [EC, L], f16, kind="ExternalInput").ap()
    K = nc.dram_tensor("K", [G * L, GC], f16, kind="ExternalInput").ap()
    V = nc.dram_tensor("V", [G * L, VW], f16, kind="ExternalInput").ap()
    O = nc.dram_tensor("O", [P, G * L], f16, kind="ExternalOutput").ap()

    with tile.TileContext(nc) as tc, ExitStack() as ctx:
        singles = ctx.enter_context(tc.tile_pool(name="singles", bufs=1))
        ld = ctx.enter_context(tc.tile_pool(name="ld", bufs=3))
        vb = ctx.enter_context(tc.tile_pool(name="vb", bufs=3))
        ph = ctx.enter_context(tc.tile_pool(name="ph", bufs=3))
        rcp = ctx.enter_context(tc.tile_pool(name="rcp", bufs=6))
        ob = ctx.enter_context(tc.tile_pool(name="ob", bufs=3))
        pn = ctx.enter_context(tc.tile_pool(name="pn", bufs=4, space="PSUM"))
        pk = ctx.enter_context(tc.tile_pool(name="pk", bufs=1, space="PSUM"))

        sig_bias = singles.tile([P, 1], f32)
        nc.vector.memset(sig_bias, -4.102)

        # Block-diagonal kv operand per head pair: rows 0:64 cols 0:65 hold
        # kv_ext of the even head, rows 64:128 cols 65:130 the odd head.
        kv_bd = singles.tile([P, G, VW], f16)
        nc.vector.memset(kv_bd, 0.0)

        # Full-bank PSUM tiles (no matmul output may straddle a bank).
        kv_ps = [pk.tile([P, 512], f32, tag=f"kv{g}", name=f"kv{g}")
                 for g in range(G)]

        # Whole-pair raw-Q and phi_Q resident buffers (8 KiB/part each).
        q_raw = [singles.tile([P, L], f16, tag=f"qr{g}", name=f"qr{g}")
                 for g in range(G)]
        phiq = [singles.tile([P, L], f16, tag=f"pq{g}", name=f"pq{g}")
                for g in range(G)]

        def emit_q_load(g):
            # 512-element inner dim forces 1 KiB descriptors so the Q
            # stream cannot out-compete K/V in DMA round-robin.
            nc.scalar.dma_start(
                out=q_raw[g].rearrange("p (k e) -> p k e", e=512),
                in_=Q[g * P:(g + 1) * P, :].rearrange(
                    "p (k e) -> p k e", e=512),
            )

        def emit_q_sigmoid(g):
            nc.scalar.activation(
                out=phiq[g], in_=q_raw[g], func=SIG, bias=sig_bias,
                scale=0.6053,
            )

        def emit_kv_batch(g, ib):
            """Load K batch + V batch, sigmoid, accumulate pair kv."""
            rows = slice(g * L + ib * TBS * P * SUB,
                         g * L + (ib + 1) * TBS * P * SUB)
            k_raw = ld.tile([P, TBS, SUB, GC], f16, tag="kraw", name="k_raw")
            nc.sync.dma_start(
                out=k_raw,
                in_=K[rows, :].rearrange("(t p s) e -> p t s e", p=P, s=SUB),
            )
            vt = vb.tile([P, TBS, SUB, VW], f16, tag="v", name="v")
            nc.sync.dma_start(
                out=vt,
                in_=V[rows, :].rearrange("(t p s) e -> p t s e", p=P, s=SUB),
            )
            phiK = ph.tile([P, TBS, SUB, GC], f16, tag="phiK", name="phiK")
            nc.scalar.activation(
                out=phiK, in_=k_raw, func=SIG, bias=sig_bias, scale=0.6053
            )
            for t in range(TBS):
                for s in range(SUB):
                    nc.tensor.matmul(
                        out=kv_ps[g][:, 0:VW],
                        lhsT=phiK[:, t, s, :],
                        rhs=vt[:, t, s, :],
                        start=(ib == 0 and t == 0 and s == 0),
                        stop=(ib == NBS - 1 and t == TBS - 1
                              and s == SUB - 1),
                    )

        def emit_kv_finish(g):
            """Pack the pair's kv PSUM tile into the block-diag operand."""
            nc.vector.tensor_copy(
                out=kv_bd[0:D, g, 0:D], in_=kv_ps[g][0:D, 0:D])
            nc.vector.tensor_copy(
                out=kv_bd[0:D, g, D:W], in_=kv_ps[g][0:D, 2 * D:2 * D + 1])
            nc.vector.tensor_copy(
                out=kv_bd[D:P, g, W:W + D], in_=kv_ps[g][D:P, D:2 * D])
            nc.vector.tensor_copy(
                out=kv_bd[D:P, g, W + D:VW], in_=kv_ps[g][D:P, 2 * D:2 * D + 1])

        def emit_q_piece(g, qb):
            """16 matmuls + batched div + one store (no Scalar dep)."""
            out_t = ob.tile([P, QB], f16, tag="outt", name="out_t")
            nqk = QB // P               # 16
            qk = 0
            while qk < nqk:
                nb = min(3, nqk - qk)   # triples, remainder 1
                num = pn.tile([P, 3, VW], f32, tag="num", name="num")
                for i in range(nb):
                    nc.tensor.matmul(
                        out=num[:, i, :],
                        lhsT=phiq[g][:, qb * QB + (qk + i) * P:
                                     qb * QB + (qk + i + 1) * P],
                        rhs=kv_bd[:, g, :],
                    )
                r = rcp.tile([P, 3, 2], f32, tag="r", name="r")
                den = bass.AP(
                    tensor=num.tensor, offset=num.offset + D,
                    ap=[num.ap[0], [VW, nb], [W, 2]],
                )
                nc.vector.reciprocal(out=r[:, 0:nb, :], in_=den)
                nums = bass.AP(
                    tensor=num.tensor, offset=num.offset,
                    ap=[num.ap[0], [VW, nb], [W, 2], [1, D]],
                )
                r_bc = bass.AP(
                    tensor=r.tensor, offset=r.offset,
                    ap=[r.ap[0], [2, nb], [1, 2], [0, D]],
                )
                nc.vector.tensor_tensor(
                    out=out_t[:, qk * P:(qk + nb) * P].rearrange(
                        "p (a b d) -> p a b d", a=nb, b=2),
                    in0=nums, in1=r_bc, op=mybir.AluOpType.mult,
                )
                qk += nb
            obase = g * L + qb * QB
            nc.gpsimd.dma_start(
                out=O[:, obase:obase + QB].rearrange("p (k e) -> p k e", e=512),
                in_=out_t.rearrange("p (k e) -> p k e", e=512),
            )

        # ---- software-pipelined emission: pair g's kv streams while
        # pair g-1's Q phase computes ----
        emit_q_load(0)
        emit_q_load(1)
        for ib in range(NBS):
            emit_kv_batch(0, ib)
            if ib == 0:
                emit_q_load(2)
        emit_q_sigmoid(0)
        emit_kv_finish(0)
        for g in range(1, G):
            for ib in range(NBS):
                emit_kv_batch(g, ib)
                if g == 1 and ib == 0:
                    emit_q_load(3)
                if ib == 1:
                    emit_q_sigmoid(g)
                if ib % 2 == 1:
                    emit_q_piece(g - 1, (ib - 1) // 2)
            emit_kv_finish(g)
        for qb in range(NQB):
            emit_q_piece(G - 1, qb)

    nc.compile()
    return nc


def _get_nc():
    if "nc" not in _CACHE:
        _CACHE["nc"] = _build_nc()
    return _CACHE["nc"]


def _shard_q(arr):
    """Full [B, L, E] f32 -> per-core transposed [512, L] fp16 slices."""
    out = []
    for c in range(N_CORES):
        b, g = divmod(c, 2)
        out.append(np.ascontiguousarray(
            arr[b, :, g * EC:(g + 1) * EC].T.astype(np.float16)))
    return out


def _shard_k(arr):
    """Full [B, L, E] f32 -> per-core pair-major [4L, 128] fp16."""
    out = []
    for c in range(N_CORES):
        b, g = divmod(c, 2)
        sl = arr[b, :, g * EC:(g + 1) * EC].astype(np.float16)
        out.append(np.ascontiguousarray(
            np.concatenate([sl[:, pg * GC:(pg + 1) * GC] for pg in range(G)],
                           axis=0)))
    return out


def _shard_v(arr):
    """Full [B, L, E] f32 -> per-core pair-major [4L, 130] fp16 with
    ones baked into columns 128:130."""
    out = []
    for c in range(N_CORES):
        b, g = divmod(c, 2)
        sl = arr[b, :, g * EC:(g + 1) * EC].astype(np.float16)
        st = np.ones((G * L, VW), dtype=np.float16)
        for pg in range(G):
            st[pg * L:(pg + 1) * L, 0:P] = sl[:, pg * P:(pg + 1) * P]
        out.append(st)
    return out


def _unshard_o(o):
    """Per-core [128, 4L] fp16 -> [L, EC] f32 core slice."""
    blocks = o.reshape(P, G, NQB, QB // P, P)   # [p, pg, qb, qk, e]
    # q = qb*QB + qk*P + p
    perm = blocks.transpose(1, 2, 3, 0, 4).reshape(G, L, P)
    return np.concatenate(list(perm), axis=1).astype(np.float32)


def run_sharded(in_maps, trace=False, trace_cores=None):
    from concourse.bass_utils import run_bass_kernel_spmd

    nc = _get_nc()
    kwargs = {}
    if trace:
        kwargs = dict(trace=True, trace_cores=trace_cores or [0])
    return run_bass_kernel_spmd(nc, in_maps, core_ids=list(range(N_CORES)), **kwargs)


def kernel(**inputs):
    Q = np.asarray(inputs["Q"], dtype=np.float32)
    K = np.asarray(inputs["K"], dtype=np.float32)
    V = np.asarray(inputs["V"], dtype=np.float32)
    qs, ks, vs = _shard_q(Q), _shard_k(K), _shard_v(V)
    in_maps = [{"Q": qs[c], "K": ks[c], "V": vs[c]} for c in range(N_CORES)]
    res = run_sharded(in_maps)
    out = np.empty((B, L, E), dtype=np.float32)
    for c in range(N_CORES):
        b, g = divmod(c, 2)
        out[b, :, g * EC:(g + 1) * EC] = _unshard_o(res.results[c]["O"])
    return out
